# revision 2
# baseline (speedup 1.0000x reference)
"""Distributed GAT layer kernel for 8 TRN2 NeuronCores.

Row-parallel over the 4096 query nodes: core k owns rows [512k, 512(k+1)).

Score algebra: softmax is invariant to per-(q,h) scaling, so with
  lrelu(x) = 0.2x + 0.8 relu(x)  and  x = sl[q] + sr[j]:
  exp(lrelu(x)) / exp(0.2 sl[q]) = max(exp(0.2 sr[j]),
                                       exp(sr[j]) * exp(0.8 sl[q]))
i.e. P[j,q] = mask01[j,q] * max(u[j], v[j] * w[q]) with
  u = exp(0.2 sr), v = exp(sr) (per-partition scalars key-major) and
  w = exp(0.8 sl) (a broadcast row).  The entire scores+softmax hot loop
becomes: one fused tensor_scalar (mult+max, 4x DVE mode) per head plus a
single mask tensor_tensor (2x) with the {0,1} mask broadcast across heads.
No exp/lrelu in the loop; exp runs once on 1-D arrays on ACT.

The {0,1} fp16 mask transposes come from: DRAM->DRAM casting DMA
(int32 A -> fp16, gpsimd SWDGE) into a key-major staging buffer, then one
crossbar-transpose DMA per key tile.  All 32 transposed tiles stay
resident in SBUF so the whole mask pipeline overlaps the AllGather.
"""

import sys

sys.path.insert(0, "/opt/trn_rl_repo")

import numpy as np

N = 4096
D = 512
HEADS = 4
DK = 128
NCORES = 8
CQ = N // NCORES          # query rows per core = 512
NQT = CQ // 128           # 4 query tiles per core
NJT = N // 128            # 32 key tiles

_CACHE = {}


def _build(debug=False, mock_cc=False):
    import concourse.bass as bass
    import concourse.mybir as mybir
    from concourse import bacc, tile

    f32 = mybir.dt.float32
    fp16 = mybir.dt.float16
    i32 = mybir.dt.int32
    AF = mybir.ActivationFunctionType
    OP = mybir.AluOpType

    nc = bacc.Bacc(
        "TRN2",
        target_bir_lowering=False,
        debug=debug,
        enable_asserts=True,
        num_devices=NCORES,
    )

    H = nc.dram_tensor("H", [CQ, D], f32, kind="ExternalInput")
    A = nc.dram_tensor("A", [CQ, N], i32, kind="ExternalInput")
    W = nc.dram_tensor("W", [D, D], f32, kind="ExternalInput")
    WLR = nc.dram_tensor("wlr", [D, 8], f32, kind="ExternalInput")
    IDENT = nc.dram_tensor("ident", [128, 128], f32, kind="ExternalInput")
    SEL = nc.dram_tensor("sel", [8, HEADS, 128], f32, kind="ExternalInput")
    OUT = nc.dram_tensor("out", [CQ, D], f32, kind="ExternalOutput")

    with tile.TileContext(nc) as tc:
        with (
            tc.tile_pool(name="const", bufs=1) as constp,
            tc.tile_pool(name="stage", bufs=1) as stagep,
            tc.tile_pool(name="sp", bufs=3) as spp,
            tc.tile_pool(name="pp", bufs=3) as ppp,
            tc.tile_pool(name="outp", bufs=2) as outp,
            tc.tile_pool(name="dram", bufs=1, space="DRAM") as dramp,
        ):
            agin = dramp.tile([CQ, 520], fp16, tag="agin")
            agout = dramp.tile(
                [N, 520], fp16, tag="agout",
                addr_space="Local" if mock_cc else "Shared",
            )
            m5d = dramp.tile([NJT, CQ, 128], fp16, tag="m5d")

            # ---------------- Mask pipeline ----------------
            # D2D casting DMA per 512-key panel (int32 -> fp16 {0,1}),
            # then one transpose DMA per key tile.  Emitted first so the
            # DMAs flow under stage A and the AllGather.
            AT = constp.tile([128, NJT, CQ], fp16, tag="AT")
            for pn in range(8):
                nc.gpsimd.dma_start(
                    m5d[pn * 4:(pn + 1) * 4],
                    A[:, pn * 512:(pn + 1) * 512]
                    .rearrange("q (jj c) -> jj q c", jj=4),
                )
                for jj in range(4):
                    jt = pn * 4 + jj
                    nc.scalar.dma_start_transpose(AT[:, jt, :], m5d[jt])

            # ---------------- Stage A: projections ----------------
            hst = stagep.tile([128, NQT, D], f32, tag="hst")
            nc.sync.dma_start(hst[:], H.rearrange("(a p) d -> p a d", p=128))
            hbf = stagep.tile([128, NQT, D], fp16, tag="hbf")
            nc.vector.tensor_copy(hbf[:], hst[:])

            wst = stagep.tile([128, 4, D], f32, tag="wst")
            nc.sync.dma_start(wst[:], W.rearrange("(a p) d -> p a d", p=128))
            WB = constp.tile([128, 4, D], fp16, tag="WB")
            nc.vector.tensor_copy(WB[:], wst[:])
            lst = stagep.tile([128, 4, 8], f32, tag="lst")
            nc.sync.dma_start(lst[:], WLR.rearrange("(a p) d -> p a d", p=128))
            WLRB = constp.tile([128, 4, 8], fp16, tag="WLRB")
            nc.vector.tensor_copy(WLRB[:], lst[:])

            idb = constp.tile([128, 128], f32, tag="idb")
            nc.sync.dma_start(idb[:], IDENT[:])
            idb16 = constp.tile([128, 128], fp16, tag="idb16")
            nc.vector.tensor_copy(idb16[:], idb[:])

            slsr_sb = constp.tile([128, 4, 8], f32, tag="slsr_sb")

            with tc.tile_pool(name="psa", bufs=2, space="PSUM") as psap:
                # H.T via PE transposes
                HT = constp.tile([128, 4, CQ], fp16, tag="HT")  # [c, ct, q]
                for qt in range(NQT):
                    for ct in range(4):
                        pht = psap.tile([128, 128], fp16, tag="pt", name="pht")
                        nc.tensor.transpose(
                            pht[:], hbf[:, qt, ct * 128:(ct + 1) * 128],
                            idb16[:],
                        )
                        nc.vector.tensor_copy(
                            HT[:, ct, qt * 128:(qt + 1) * 128], pht[:]
                        )
                for qt in range(NQT):
                    ps = psap.tile([128, D], f32, tag="ps")
                    for ct in range(4):
                        nc.tensor.matmul(
                            ps[:],
                            HT[:, ct, qt * 128:(qt + 1) * 128],
                            WB[:, ct, :],
                            start=(ct == 0),
                            stop=(ct == 3),
                        )
                    pss = psap.tile([128, 8], f32, tag="pss")
                    for ct in range(4):
                        nc.tensor.matmul(
                            pss[:],
                            HT[:, ct, qt * 128:(qt + 1) * 128],
                            WLRB[:, ct, :],
                            start=(ct == 0),
                            stop=(ct == 3),
                        )
                    whbf = stagep.tile([128, 520], fp16, tag="whbf")
                    nc.vector.tensor_copy(whbf[:, 0:512], ps[:])
                    nc.vector.tensor_copy(whbf[:, 512:520], pss[:])
                    nc.vector.tensor_copy(slsr_sb[:, qt, :], pss[:])
                    nc.sync.dma_start(agin[qt * 128:(qt + 1) * 128, :], whbf[:])

                # w = exp(0.8*sl) broadcast rows, one per head
                slsrT = constp.tile([8, 4, 128], f32, tag="slsrT")
                for qt in range(NQT):
                    pst = psap.tile([8, 128], f32, tag="pt", name="pst")
                    nc.tensor.transpose(pst[:], slsr_sb[:, qt, :], idb[:])
                    nc.vector.tensor_copy(slsrT[:, qt, :], pst[:])
                wexpT = constp.tile([8, 4, 128], f32, tag="wexpT")
                nc.scalar.activation(wexpT[:], slsrT[:], AF.Exp, scale=0.8)
                wexpT16 = constp.tile([8, 4, 128], fp16, tag="wexpT16")
                nc.vector.tensor_copy(wexpT16[:], wexpT[:])
                self_f = stagep.tile([8, HEADS, 128], f32, tag="self_f")
                nc.sync.dma_start(self_f[:], SEL[:])
                sel = constp.tile([8, HEADS, 128], fp16, tag="sel")
                nc.vector.tensor_copy(sel[:], self_f[:])
                WBC = constp.tile([128, HEADS, CQ], fp16, tag="WBC")
                for h in range(HEADS):
                    psb = psap.tile([128, CQ], f32, tag="ps", name="psb")
                    nc.tensor.matmul(
                        psb[:], sel[:, h, :], wexpT16[:], start=True, stop=True
                    )
                    nc.vector.tensor_copy(WBC[:, h, :], psb[:])

            # ---------------- Stage B: AllGather ----------------
            if mock_cc:
                nc.sync.dma_start(agout[0:CQ, :], agin[:])
            else:
                nc.gpsimd.collective_compute(
                    "AllGather",
                    OP.bypass,
                    replica_groups=[list(range(NCORES))],
                    ins=[agin[:]],
                    outs=[agout[:]],
                )

            # ---------------- Post-AG prep ----------------
            srsl = constp.tile([128, NJT, 8], fp16, tag="srsl")
            nc.sync.dma_start(
                srsl[:],
                agout[:, 512:520].rearrange("(jt p) c -> p jt c", p=128),
            )
            # u = exp(0.2*sr), v = exp(sr): per-partition scalars, key-major
            u32 = constp.tile([128, NJT, 4], f32, tag="u32")
            nc.scalar.activation(u32[:], srsl[:, :, 4:8], AF.Exp, scale=0.2)
            v32 = constp.tile([128, NJT, 4], f32, tag="v32")
            nc.scalar.activation(v32[:], srsl[:, :, 4:8], AF.Exp)

            # Wh_aug [j, jt, h, dk+1] with ones column for the denominator
            WHA = constp.tile([128, NJT, HEADS, DK + 1], fp16, tag="WHA")
            nc.gpsimd.memset(WHA[:, :, :, DK:DK + 1], 1.0)

            def emit_wha_chunk(jc):
                for h in range(HEADS):
                    nc.sync.dma_start(
                        WHA[:, jc:jc + 8, h, 0:DK],
                        agout[jc * 128:(jc + 8) * 128, h * DK:(h + 1) * DK]
                        .rearrange("(jt p) d -> p jt d", p=128),
                    )

            # ---------------- Stage C: attention ----------------
            with tc.tile_pool(name="psc", bufs=1, space="PSUM") as pscp:
                accs = [
                    pscp.tile(
                        [128, 2, DK + 1], f32, tag=f"acc{i}", name=f"acc{i}"
                    )
                    for i in range(8)
                ]
                for acc in accs:
                    nc.vector.memset(acc[:], 0.0)

                emit_wha_chunk(0)
                emit_wha_chunk(8)

                for jt in range(NJT):
                    # P[j,q] = mask01 * max(u[j], v[j]*w[q]) per head
                    sp = spp.tile([128, HEADS, CQ], fp16, tag="sp")
                    for h in range(HEADS):
                        nc.vector.tensor_scalar(
                            sp[:, h, :], WBC[:, h, :],
                            v32[:, jt, h:h + 1], u32[:, jt, h:h + 1],
                            op0=OP.mult, op1=OP.max,
                        )
                    pp = ppp.tile([128, HEADS, CQ], fp16, tag="pp")
                    nc.vector.tensor_tensor(
                        pp[:], sp[:],
                        AT[:, jt:jt + 1, :].to_broadcast([128, HEADS, CQ]),
                        op=OP.mult,
                    )
                    for qt in range(NQT):
                        for h in range(HEADS):
                            acc = accs[qt * 2 + h // 2]
                            nc.tensor.matmul(
                                acc[:, h % 2, :],
                                pp[:, h, qt * 128:(qt + 1) * 128],
                                WHA[:, jt, h, :],
                                start=False,
                                stop=False,
                                skip_group_check=True,
                            )
                    if jt % 8 == 2 and jt + 14 < NJT:
                        emit_wha_chunk(jt + 14)

                # ---------------- Epilogue: 1/D scale + ELU ----------------
                for qt in range(NQT):
                    rec = outp.tile([128, HEADS], f32, tag="rec")
                    o = outp.tile([128, HEADS, DK], f32, tag="o")
                    for h in range(HEADS):
                        acc = accs[qt * 2 + h // 2]
                        nc.vector.reciprocal(
                            rec[:, h:h + 1], acc[:, h % 2, DK:DK + 1]
                        )
                        nc.vector.tensor_scalar(
                            o[:, h, :], acc[:, h % 2, 0:DK], rec[:, h:h + 1],
                            None, op0=OP.mult,
                        )
                    m = outp.tile([128, HEADS, DK], f32, tag="m")
                    nc.vector.tensor_scalar(m[:], o[:], 0.0, None, op0=OP.min)
                    e = outp.tile([128, HEADS, DK], f32, tag="e")
                    nc.scalar.activation(e[:], m[:], AF.Exp)
                    r = outp.tile([128, HEADS, DK], f32, tag="r")
                    nc.vector.tensor_scalar(r[:], o[:], 0.0, None, op0=OP.max)
                    of = outp.tile([128, HEADS, DK], f32, tag="of")
                    nc.vector.scalar_tensor_tensor(
                        of[:], e[:], 1.0, r[:], op0=OP.subtract, op1=OP.add
                    )
                    nc.sync.dma_start(OUT[qt * 128:(qt + 1) * 128, :], of[:])

    return nc


def _prep_inputs(H, A, W, a_l, a_r):
    wl = np.einsum("chd,hd->ch", W.reshape(D, HEADS, DK), a_l).astype(np.float32)
    wr = np.einsum("chd,hd->ch", W.reshape(D, HEADS, DK), a_r).astype(np.float32)
    wlr = np.ascontiguousarray(np.concatenate([wl, wr], axis=1))
    ident = np.eye(128, dtype=np.float32)
    sel = np.zeros((8, HEADS, 128), dtype=np.float32)
    for h in range(HEADS):
        sel[h, h, :] = 1.0
    in_maps = []
    idx = np.arange(CQ)
    for k in range(NCORES):
        rows = slice(k * CQ, (k + 1) * CQ)
        Ak = np.ascontiguousarray(A[rows]).copy()
        Ak[idx, k * CQ + idx] = 1  # self loops always allowed
        in_maps.append(
            {
                "H": np.ascontiguousarray(H[rows]).astype(np.float32),
                "A": Ak.astype(np.int32),
                "W": np.ascontiguousarray(W).astype(np.float32),
                "wlr": wlr,
                "ident": ident,
                "sel": sel,
            }
        )
    return in_maps


def kernel(H, A, W, a_l, a_r, _trace=False):
    from concourse.bass_utils import run_bass_kernel_spmd

    H = np.asarray(H, dtype=np.float32)
    A = np.asarray(A, dtype=np.int32)
    W = np.asarray(W, dtype=np.float32)
    a_l = np.asarray(a_l, dtype=np.float32)
    a_r = np.asarray(a_r, dtype=np.float32)

    if "nc" not in _CACHE:
        nc = _build()
        nc.finalize()  # Bacc register allocation; required for the PJRT path
        _CACHE["nc"] = nc
    nc = _CACHE["nc"]

    in_maps = _prep_inputs(H, A, W, a_l, a_r)
    kw = {}
    if _trace:
        import tempfile

        kw["tmpdir"] = tempfile.mkdtemp(prefix="gat_trace_")
        _CACHE["tmpdir"] = kw["tmpdir"]
    res = run_bass_kernel_spmd(
        nc, in_maps, core_ids=list(range(NCORES)), trace=_trace, **kw
    )
    out = np.concatenate([res.results[k]["out"] for k in range(NCORES)], axis=0)
    if _trace:
        _CACHE["exec_time_ns"] = res.exec_time_ns
        _CACHE["profile_json"] = res.profile_json
    return out
